# revision 11
# baseline (speedup 1.0000x reference)
"""Trainium2 Bass kernel: MemModule (softmax memory attention + hard-shrink).

Computation (per token row x of length 512, memory W [2000, 512]):
    att = softmax(x @ W.T); gate at threshold 0.0025; L1-renormalize;
    y = att @ W.  Outputs y (NCHW) and att (B, 2000, H, W).

Sharding: data-parallel over batch across 8 NeuronCores (4 images each),
weight replicated.

Precision: matmuls run as 3-pass split-fp16 (a = a_hi + a_lo rounded to
fp16; a@b ~ a_hi@b_hi + a_lo@b_hi + a_hi@b_lo, residual ~2^-22) at full
PE rate with fp32 PSUM accumulation -- fp32-faithful results (float32r is
broken in this toolchain; plain fp16 flips ~1e-2 of gate decisions near
the shrink threshold).

Layout: tokens on partitions for MM1 + softmax; the [tok, slot] -> [slot,
tok] transpose for MM2/output runs on the DMA transpose engine (fp16
hi/lo pair, reconstructed to fp32 by DVE) so the PE does matmuls only.
The tile loop is software-pipelined one deep (MM2 of tile i-1 is emitted
after MM1 of tile i) to keep the PE instruction stream gap-free.
"""

import numpy as np

import concourse.bass as bass
import concourse.mybir as mybir
import concourse.tile as tile
from concourse import bacc
from concourse.bass_utils import run_bass_kernel_spmd

# Problem geometry (hardcoded per contract)
B, C, H, W = 32, 512, 32, 32
HW = H * W                   # 1024 tokens per image
M = 2000                     # memory slots
MP = 2048                    # slots padded to 16*128
N_CORES = 8
B_LOC = B // N_CORES         # 4 images per core
P = 128
KC = C // P                  # 4 feature chunks
SC = MP // P                 # 16 slot chunks
NT = HW // P                 # 8 token tiles per image
THRES = 0.0025
EPS = 1e-12
F32 = mybir.dt.float32
F16 = mybir.dt.float16
AF = mybir.ActivationFunctionType
OP = mybir.AluOpType

LAST_EXEC_NS = None
LAST_RESULTS = None
TRACE = False


def _build_body(tc, x_h, x_l, w_h, w_l, wt_h, wt_l, y_d, att_d):
    import contextlib

    nc = tc.nc
    ctx = contextlib.ExitStack()
    with ctx:
        const = ctx.enter_context(tc.tile_pool(name="const", bufs=1))
        xpool = ctx.enter_context(tc.tile_pool(name="xp", bufs=3))
        epool = ctx.enter_context(tc.tile_pool(name="ep", bufs=3))
        mepool = ctx.enter_context(tc.tile_pool(name="mep", bufs=3))
        aqp = ctx.enter_context(tc.tile_pool(name="aqp", bufs=2))
        aqtp = ctx.enter_context(tc.tile_pool(name="aqt", bufs=2))
        attTp = ctx.enter_context(tc.tile_pool(name="atp", bufs=2))
        ysbp = ctx.enter_context(tc.tile_pool(name="ysp", bufs=2))
        yTp = ctx.enter_context(tc.tile_pool(name="ytp", bufs=2))
        smalls = ctx.enter_context(tc.tile_pool(name="sm", bufs=6))
        ps_s = ctx.enter_context(tc.tile_pool(name="pss", bufs=3, space="PSUM"))
        ps_y = ctx.enter_context(tc.tile_pool(name="psy", bufs=2, space="PSUM"))

        # Persistent weights in SBUF (fp16 hi/lo pairs).  wth is needed by
        # the very first matmul (MM1 pass 1), so it loads first.
        wtsb = []
        for nm, src in (("wth", wt_h), ("wtl", wt_l)):
            t_ = const.tile([P, KC, MP], F16, tag=nm)  # W^T as [ki, ko, m]
            nc.sync.dma_start(t_[:], src.rearrange("(ko ki) m -> ki ko m", ki=P))
            wtsb.append(t_)
        wsb = []
        for nm, src in (("wh", w_h), ("wl", w_l)):
            t_ = const.tile([P, SC, C], F16, tag=nm)   # W as [si, so, c]
            nc.sync.dma_start(t_[:], src.rearrange("(so si) c -> si so c", si=P))
            wsb.append(t_)

        def stage1(b, t):
            """x load, MM1, softmax + gate, hi/lo split, DMA-transpose."""
            ws = bass.ts(t, P)
            xTh = xpool.tile([P, KC, P], F16, tag="xh")
            nc.sync.dma_start(
                xTh[:], x_h[b, :, ws].rearrange("(ko ki) w -> ki ko w", ki=P)
            )
            xTl = xpool.tile([P, KC, P], F16, tag="xl")
            nc.sync.dma_start(
                xTl[:], x_l[b, :, ws].rearrange("(ko ki) w -> ki ko w", ki=P)
            )

            mm1 = ((xTh, wtsb[0]), (xTl, wtsb[0]), (xTh, wtsb[1]))
            e = epool.tile([P, MP], F32)
            sa = smalls.tile([P, 1], F32, tag="sa")
            sb = smalls.tile([P, 1], F32, tag="sb")
            for h, acc in ((0, sa), (1, sb)):
                s_ps = ps_s.tile([P, 1024], F32, tag="sps")
                for p_, (xt, wt) in enumerate(mm1):
                    for k in range(KC):
                        for n in range(2):
                            nc.tensor.matmul(
                                s_ps[:, n * 512 : (n + 1) * 512],
                                lhsT=xt[:, k, :],
                                rhs=wt[
                                    :, k,
                                    h * 1024 + n * 512 : h * 1024 + (n + 1) * 512,
                                ],
                                start=(p_ == 0 and k == 0),
                                stop=(p_ == 2 and k == KC - 1),
                            )
                # exp (no max-subtraction: |scores| < ~6) + row sum
                nc.scalar.activation(
                    e[:, h * 1024 : (h + 1) * 1024], s_ps[:], AF.Exp,
                    accum_out=acc[:],
                )

            # tS = (sa + sb - 48) * THRES   (48 = padded slots, exp(0) each)
            ts0 = smalls.tile([P, 1], F32, tag="ts0")
            nc.vector.tensor_tensor(ts0[:], sa[:], sb[:], op=OP.add)
            tS = smalls.tile([P, 1], F32, tag="tS")
            nc.vector.tensor_scalar(tS[:], ts0[:], -48.0, THRES, op0=OP.add, op1=OP.mult)

            # fused gate: me = (e > tS) * e ; s2 = sum(me)
            me = mepool.tile([P, MP], F32)
            s2 = smalls.tile([P, 1], F32, tag="s2")
            nc.vector.scalar_tensor_tensor(
                me[:], in0=e[:], scalar=tS[:], in1=e[:],
                op0=OP.is_gt, op1=OP.mult, accum_out=s2[:],
            )

            # c = 1 / max(s2, S * EPS);  S*EPS = tS * (EPS/THRES)
            seps = smalls.tile([P, 1], F32, tag="seps")
            nc.vector.tensor_scalar(seps[:], tS[:], EPS / THRES, None, op0=OP.mult)
            s2c = smalls.tile([P, 1], F32, tag="s2c")
            nc.vector.tensor_tensor(s2c[:], s2[:], seps[:], op=OP.max)
            cinv = smalls.tile([P, 1], F32, tag="cinv")
            nc.vector.reciprocal(cinv[:], s2c[:])

            # att hi/lo fp16 split of me*c (att_final never materialized fp32)
            aqh = aqp.tile([P, MP], F16, tag="aqh")
            nc.scalar.mul(aqh[:], me[:], cinv[:])
            aql = aqp.tile([P, MP], F16, tag="aql")
            nc.vector.scalar_tensor_tensor(
                aql[:], in0=me[:], scalar=cinv[:], in1=aqh[:],
                op0=OP.mult, op1=OP.subtract,
            )

            # DMA-transpose hi/lo -> [si, so, tok]
            aqhT = aqtp.tile([P, SC, P], F16, tag="aqhT")
            nc.sync.dma_start_transpose(aqhT[:], aqh[:])
            aqlT = aqtp.tile([P, SC, P], F16, tag="aqlT")
            nc.sync.dma_start_transpose(aqlT[:], aql[:])
            return ws, aqhT, aqlT

        def stage2(st):
            """att reconstruct + DMA out, MM2, y split/transpose + DMA out."""
            b, ws, aqhT, aqlT = st
            attT = attTp.tile([P, SC, P], F32)
            nc.vector.tensor_tensor(attT[:], aqhT[:], aqlT[:], op=OP.add)
            nc.sync.dma_start(
                att_d[b, 0:1920, ws].rearrange("(so si) w -> si so w", si=P),
                attT[:, 0:15, :],
            )
            nc.sync.dma_start(att_d[b, 1920:2000, ws], attT[0:80, 15, :])

            mm2 = ((aqhT, wsb[0]), (aqlT, wsb[0]), (aqhT, wsb[1]))
            y_ps = ps_y.tile([P, C], F32)
            for p_, (aq, wb) in enumerate(mm2):
                for j in range(SC):
                    nc.tensor.matmul(
                        y_ps[:],
                        lhsT=aq[:, j, :],
                        rhs=wb[:, j, :],
                        start=(p_ == 0 and j == 0),
                        stop=(p_ == 2 and j == SC - 1),
                    )
            # y fp16 hi/lo split -> DMA-transpose -> reconstruct [c, tok]
            yh = ysbp.tile([P, C], F16, tag="yh")
            nc.scalar.copy(yh[:], y_ps[:])
            yl = ysbp.tile([P, C], F16, tag="yl")
            nc.vector.scalar_tensor_tensor(
                yl[:], in0=y_ps[:], scalar=1.0, in1=yh[:],
                op0=OP.mult, op1=OP.subtract,
            )
            yhT = yTp.tile([P, KC, P], F16, tag="yhT")
            nc.sync.dma_start_transpose(yhT[:], yh[:])
            ylT = yTp.tile([P, KC, P], F16, tag="ylT")
            nc.sync.dma_start_transpose(ylT[:], yl[:])
            yT = yTp.tile([P, KC, P], F32, tag="yT")
            nc.vector.tensor_tensor(yT[:], yhT[:], ylT[:], op=OP.add)
            nc.sync.dma_start(
                y_d[b, :, ws].rearrange("(fo fi) w -> fi fo w", fi=P), yT[:]
            )

        # software pipeline: stage2(i-1) is emitted after stage1(i)
        pending = None
        for b in range(B_LOC):
            for t in range(NT):
                ws, aqhT, aqlT = stage1(b, t)
                if pending is not None:
                    stage2(pending)
                pending = (b, ws, aqhT, aqlT)
        stage2(pending)


_CACHE = {}


def _get_nc():
    if "nc" not in _CACHE:
        nc = bacc.Bacc("TRN2", target_bir_lowering=False, debug=False, num_devices=N_CORES)
        xh_d = nc.dram_tensor("xh", (B_LOC, C, HW), F16, kind="ExternalInput")
        xl_d = nc.dram_tensor("xl", (B_LOC, C, HW), F16, kind="ExternalInput")
        wh_d = nc.dram_tensor("wh", (MP, C), F16, kind="ExternalInput")
        wl_d = nc.dram_tensor("wl", (MP, C), F16, kind="ExternalInput")
        wth_d = nc.dram_tensor("wth", (C, MP), F16, kind="ExternalInput")
        wtl_d = nc.dram_tensor("wtl", (C, MP), F16, kind="ExternalInput")
        y_d = nc.dram_tensor("y", (B_LOC, C, HW), F32, kind="ExternalOutput")
        att_d = nc.dram_tensor("att", (B_LOC, M, HW), F32, kind="ExternalOutput")
        with tile.TileContext(nc) as tc:
            _build_body(
                tc, xh_d.ap(), xl_d.ap(), wh_d.ap(), wl_d.ap(),
                wth_d.ap(), wtl_d.ap(), y_d.ap(), att_d.ap(),
            )
        nc.compile()
        _CACHE["nc"] = nc
    return _CACHE["nc"]


def _split16(a):
    hi = a.astype(np.float16)
    lo = (a - hi.astype(np.float32)).astype(np.float16)
    return hi, lo


def kernel(x, weight):
    global LAST_EXEC_NS, LAST_RESULTS
    x = np.asarray(x, dtype=np.float32)
    weight = np.asarray(weight, dtype=np.float32)

    x3 = np.ascontiguousarray(x.reshape(B, C, HW))
    xh, xl = _split16(x3)
    w_pad = np.zeros((MP, C), np.float32)
    w_pad[:M] = weight
    wh, wl = _split16(w_pad)
    wt_pad = np.ascontiguousarray(w_pad.T)
    wth, wtl = _split16(wt_pad)

    in_maps = [
        {
            "xh": np.ascontiguousarray(xh[k * B_LOC : (k + 1) * B_LOC]),
            "xl": np.ascontiguousarray(xl[k * B_LOC : (k + 1) * B_LOC]),
            "wh": wh, "wl": wl, "wth": wth, "wtl": wtl,
        }
        for k in range(N_CORES)
    ]
    nc = _get_nc()
    res = run_bass_kernel_spmd(
        nc, in_maps, core_ids=list(range(N_CORES)), trace=TRACE
    )
    LAST_RESULTS = res
    LAST_EXEC_NS = res.exec_time_ns
    y = np.concatenate([r["y"] for r in res.results], axis=0).reshape(B, C, H, W)
    att = np.concatenate([r["att"] for r in res.results], axis=0).reshape(B, M, H, W)
    return (y, att)
